# revision 17
# baseline (speedup 1.0000x reference)
"""Multi-head attention (B=2, S=2048, E=1024, H=16) on 8 TRN2 NeuronCores.

Sharding: data-parallel over batch (2) x tensor-parallel over head groups (4).
Core c = b*4 + g handles batch b, heads [4g, 4g+4), channel slice [256g, 256g+256).

No collectives: each core computes its partial out-projection; the host sums the
4 head-group partials per batch and adds bo. Attention weights are written per
head directly in the reference [q, k] layout.

Device pipeline per core (bf16 compute, fp32 PSUM accumulation):
  phase 0: load pre-transposed inputs, project qp/kp/vp (transposed layout
           [channel, token]), PE-transpose vp to token-major for AV.
  per head:
    pass C: scores s[q,k] (q on partitions), exp on ScalarE with accum_out row
            sums -> Z, reciprocal, normalize, DMA weights out.
    pass A: scores st[k,q] (k on partitions), exp -> st bf16 blocks.
    pass B: AV matmuls (U_T[dh,q] unnormalized) + recip row transpose via PE.
  tail:    broadcast recip rows over partitions (DMA), normalize U_T,
           out-projection through Wo shard, DMA partial out.
"""

import sys

for _p in ("/opt/trn_rl_repo",):
    if _p not in sys.path:
        sys.path.insert(0, _p)

import numpy as np
import ml_dtypes

B, S, E, H = 2, 2048, 1024, 16
DH = E // H
SCALE = DH ** -0.5
HPG = 4            # heads per group (per core)
CS = 256           # channel slice width per core
NCORES = 8
BF = ml_dtypes.bfloat16

_CACHE = {}


def _build_nc():
    import concourse.bass as bass
    import concourse.tile as tile
    from concourse import bacc, mybir
    from concourse.masks import make_identity

    DT = mybir.dt.bfloat16
    F32 = mybir.dt.float32
    Exp = mybir.ActivationFunctionType.Exp

    nc = bacc.Bacc(None)

    qT_ext = nc.declare_dram_parameter("qT", [E, S], DT, isOutput=False)
    kT_ext = nc.declare_dram_parameter("kT", [E, S], DT, isOutput=False)
    vT_ext = nc.declare_dram_parameter("vT", [E, S], DT, isOutput=False)
    wqT_ext = nc.declare_dram_parameter("wqT", [E, CS], DT, isOutput=False)
    wkT_ext = nc.declare_dram_parameter("wkT", [E, CS], DT, isOutput=False)
    wvT_ext = nc.declare_dram_parameter("wvT", [E, CS], DT, isOutput=False)
    woT_ext = nc.declare_dram_parameter("woT", [CS, E], DT, isOutput=False)
    bqkv_ext = nc.declare_dram_parameter("bqkv", [128, 6], F32, isOutput=False)

    w_out = nc.declare_dram_parameter("w_out", [HPG, S, S], DT, isOutput=True)
    o_out = nc.declare_dram_parameter("o_out", [S, E], DT, isOutput=True)
    recip_dram = nc.dram_tensor("recip_dram", [HPG * (S // 128), 128], DT)

    EC = E // 128      # 8 contraction chunks for projections
    QC = S // 512      # 4 token chunks of 512
    QT = S // 128      # 16 token tiles of 128
    KT = S // 128      # 16 key tiles of 128

    with tile.TileContext(nc) as tc:
        consts = tc.alloc_tile_pool(name="consts", bufs=1)
        acts = tc.alloc_tile_pool(name="acts", bufs=1)
        ps_big = tc.alloc_tile_pool(name="ps_big", bufs=2, space="PSUM")
        ps_av = tc.alloc_tile_pool(name="ps_av", bufs=2, space="PSUM")
        ps_t = tc.alloc_tile_pool(name="ps_t", bufs=1, space="PSUM")
        st_pool = tc.alloc_tile_pool(name="st", bufs=3)
        w_pool = tc.alloc_tile_pool(name="w", bufs=4)
        o_pool = tc.alloc_tile_pool(name="o", bufs=2)
        small = tc.alloc_tile_pool(name="small", bufs=4)

        ident = consts.tile([128, 128], DT)
        make_identity(nc, ident)
        bqkv_sb = consts.tile([128, 6], F32)
        nc.sync.dma_start(out=bqkv_sb, in_=bqkv_ext[:, :])
        wq_sb = consts.tile([128, EC, CS], DT)
        wk_sb = consts.tile([128, EC, CS], DT)
        wv_sb = consts.tile([128, EC, CS], DT)
        nc.sync.dma_start(out=wq_sb, in_=wqT_ext[:, :].rearrange("(c p) n -> p c n", p=128))
        nc.sync.dma_start(out=wk_sb, in_=wkT_ext[:, :].rearrange("(c p) n -> p c n", p=128))
        nc.sync.dma_start(out=wv_sb, in_=wvT_ext[:, :].rearrange("(c p) n -> p c n", p=128))
        woT_sb = consts.tile([128, 2, E], DT)
        nc.sync.dma_start(out=woT_sb, in_=woT_ext[:, :].rearrange("(c p) n -> p c n", p=128))

        qpT_sb = acts.tile([128, 2, S], DT)      # [channel, token] per c-tile
        kpT_sb = acts.tile([128, 2, S], DT)
        vh_sb = acts.tile([128, KT, HPG, DH], DT)  # token-major v heads
        UT_sb = acts.tile([128, 2, S], DT)       # unnormalized attn out, transposed
        rb_sb = acts.tile([128, 2, S], DT)       # recip broadcast over partitions
        rcall_sb = acts.tile([128, HPG * (S // 128)], DT)  # recip columns

        # ---- phase 0: projections ----
        xin = tc.alloc_tile_pool(name="xin", bufs=2)
        vtmp = tc.alloc_tile_pool(name="vtmp", bufs=1)
        if True:
            vpT_sb = vtmp.tile([128, 2, S], DT)
            for (x_ext, w_sb_t, dst, bcol) in (
                (qT_ext, wq_sb, qpT_sb, 0),
                (kT_ext, wk_sb, kpT_sb, 2),
                (vT_ext, wv_sb, vpT_sb, 4),
            ):
                x_sb = xin.tile([128, EC, S], DT, tag="xT")
                nc.sync.dma_start(out=x_sb, in_=x_ext[:, :].rearrange("(c p) t -> p c t", p=128))
                for ct in range(2):
                    for qcp in range(2):
                        ps = ps_big.tile([128, 2, 512], F32, tag="ps_big")
                        for j in range(2):
                            qc = qcp * 2 + j
                            for ec in range(EC):
                                nc.tensor.matmul(
                                    ps[:, j, :],
                                    lhsT=w_sb_t[:, ec, ct * 128:(ct + 1) * 128],
                                    rhs=x_sb[:, ec, qc * 512:(qc + 1) * 512],
                                    start=(ec == 0),
                                    stop=(ec == EC - 1),
                                )
                        for j in range(2):
                            qc = qcp * 2 + j
                            nc.vector.tensor_scalar_add(
                                out=dst[:, ct, qc * 512:(qc + 1) * 512],
                                in0=ps[:, j, :],
                                scalar1=bqkv_sb[:, bcol + ct:bcol + ct + 1],
                            )
            # transpose vp: [channel, token] -> token-major head slices
            for ct in range(2):
                for tt in range(KT):
                    pst = ps_t.tile([128, 128], DT, tag="ps_t")
                    nc.tensor.transpose(pst, vpT_sb[:, ct, tt * 128:(tt + 1) * 128], ident)
                    for hh in range(2):
                        h = 2 * ct + hh
                        nc.vector.tensor_copy(
                            out=vh_sb[:, tt, h, :],
                            in_=pst[:, hh * 64:(hh + 1) * 64],
                        )
        vtmp.release()
        xin.release()

        # ---- per-head attention, processed as head pairs ----
        for hp in range(2):
            h0, h1 = 2 * hp, 2 * hp + 1

            # pass C: s[q, k] + softmax weights output (per head)
            for h in (h0, h1):
                po = (h % 2) * 64
                for qt in range(QT):
                    ps = ps_big.tile([128, 2, 512], F32, tag="ps_big")
                    ps2 = ps_big.tile([128, 2, 512], F32, tag="ps_big")
                    w_sb = w_pool.tile([128, 4, 512], DT, tag="w")
                    zparts = small.tile([128, 2], F32, tag="zp")
                    for half, psh in ((0, ps), (1, ps2)):
                        for kc2 in range(2):
                            kc = half * 2 + kc2
                            nc.tensor.matmul(
                                psh[:, kc2, :],
                                lhsT=qpT_sb[po:po + 64, hp, qt * 128:(qt + 1) * 128],
                                rhs=kpT_sb[po:po + 64, hp, kc * 512:(kc + 1) * 512],
                                start=True,
                                stop=True,
                            )
                        nc.scalar.activation(
                            out=w_sb[:, half * 2:half * 2 + 2, :],
                            in_=psh[:, :, :],
                            func=Exp,
                            accum_out=zparts[:, half:half + 1],
                        )
                    z = small.tile([128, 1], F32, tag="z")
                    nc.vector.tensor_add(z, zparts[:, 0:1], zparts[:, 1:2])
                    rc = small.tile([128, 1], F32, tag="rc")
                    nc.vector.reciprocal(rc, z)
                    nc.vector.tensor_scalar_mul(out=w_sb, in0=w_sb, scalar1=rc)
                    nc.sync.dma_start(
                        out=w_out[h, qt * 128:(qt + 1) * 128, :],
                        in_=w_sb[:, :, :],
                    )
                    nc.vector.tensor_copy(
                        out=rcall_sb[:, h * QT + qt:h * QT + qt + 1], in_=rc,
                    )

            # pass A/B: st[k, q] + AV, both heads packed on the PE array
            for qc in range(QC):
                st0 = st_pool.tile([128, KT, 512], DT, tag="st")
                st1 = st_pool.tile([128, KT, 512], DT, tag="st")
                for ktp in range(KT // 2):
                    ps = ps_big.tile([128, 2, 512], F32, tag="ps_big")
                    ps2 = ps_big.tile([128, 2, 512], F32, tag="ps_big")
                    for j in range(2):
                        kt = ktp * 2 + j
                        nc.tensor.matmul(
                            ps[:, j, :],
                            lhsT=kpT_sb[0:64, hp, kt * 128:(kt + 1) * 128],
                            rhs=qpT_sb[0:64, hp, qc * 512:(qc + 1) * 512],
                            start=True,
                            stop=True,
                        )
                        nc.tensor.matmul(
                            ps2[:, j, :],
                            lhsT=kpT_sb[64:128, hp, kt * 128:(kt + 1) * 128],
                            rhs=qpT_sb[64:128, hp, qc * 512:(qc + 1) * 512],
                            start=True,
                            stop=True,
                        )
                    nc.scalar.activation(
                        out=st0[:, ktp * 2:ktp * 2 + 2, :], in_=ps[:, :, :], func=Exp,
                    )
                    nc.scalar.activation(
                        out=st1[:, ktp * 2:ktp * 2 + 2, :], in_=ps2[:, :, :], func=Exp,
                    )
                av = ps_av.tile([128, 512], F32, tag="av")
                for kt in range(KT):
                    nc.tensor.matmul(
                        av[0:64, :],
                        lhsT=vh_sb[:, kt, h0, :],
                        rhs=st0[:, kt, :],
                        start=(kt == 0),
                        stop=(kt == KT - 1),
                    )
                    nc.tensor.matmul(
                        av[64:128, :],
                        lhsT=vh_sb[:, kt, h1, :],
                        rhs=st1[:, kt, :],
                        start=(kt == 0),
                        stop=(kt == KT - 1),
                    )
                nc.vector.tensor_copy(
                    out=UT_sb[:, hp, qc * 512:(qc + 1) * 512],
                    in_=av,
                )

            # recip rows for this pair -> DRAM -> partition broadcast -> normalize UT
            pst = ps_t.tile([32, 128], DT, tag="ps_t2")
            nc.tensor.transpose(pst, rcall_sb[:, hp * 2 * QT:(hp + 1) * 2 * QT], ident)
            rrow_sb = small.tile([32, 128], DT, tag="rrow")
            nc.vector.tensor_copy(out=rrow_sb, in_=pst)
            nc.sync.dma_start(
                out=recip_dram[hp * 2 * QT:(hp + 1) * 2 * QT, :], in_=rrow_sb[:, :],
            )
            for hh in range(2):
                h = 2 * hp + hh
                nc.gpsimd.dma_start(
                    out=rb_sb[hh * 64:hh * 64 + 64, hp, :],
                    in_=recip_dram[h * QT:(h + 1) * QT, :].rearrange("a b -> (a b)")[None, :].to_broadcast([64, S]),
                )
            nc.vector.tensor_mul(
                out=UT_sb[:, hp, :],
                in0=UT_sb[:, hp, :],
                in1=rb_sb[:, hp, :],
            )

        # ---- tail: out projection ----
        for qt in range(QT):
            ps = ps_big.tile([128, 2, 512], F32, tag="ps_big")
            for ec in range(2):
                for cc in range(2):
                    nc.tensor.matmul(
                        ps[:, ec, :],
                        lhsT=UT_sb[:, cc, qt * 128:(qt + 1) * 128],
                        rhs=woT_sb[:, cc, ec * 512:(ec + 1) * 512],
                        start=(cc == 0),
                        stop=(cc == 1),
                    )
            o_sb = o_pool.tile([128, 2, 512], DT, tag="o")
            nc.vector.tensor_copy(out=o_sb, in_=ps)
            nc.sync.dma_start(out=o_out[qt * 128:(qt + 1) * 128, :], in_=o_sb[:, :, :])

        for p in (small, o_pool, w_pool, st_pool, ps_t, ps_av, ps_big, acts, consts):
            p.release()

    nc.finalize()
    return nc


def _get_nc():
    if "nc" not in _CACHE:
        _CACHE["nc"] = _build_nc()
    return _CACHE["nc"]


def _make_in_maps(q, k, v, Wq, bq, Wk, bk, Wv, bv, Wo, bo):
    in_maps = []
    perb = []
    for b in range(B):
        perb.append((
            np.ascontiguousarray(q[b].T).astype(BF),
            np.ascontiguousarray(k[b].T).astype(BF),
            np.ascontiguousarray(v[b].T).astype(BF),
        ))
    for c in range(NCORES):
        b, g = divmod(c, HPG)
        cs = slice(g * CS, (g + 1) * CS)
        qTb, kTb, vTb = perb[b]
        bqkv = np.zeros((128, 6), np.float32)
        for col, vec in ((0, bq[cs] * SCALE), (2, bk[cs]), (4, bv[cs])):
            bqkv[:, col] = vec[:128]
            bqkv[:, col + 1] = vec[128:]
        in_maps.append({
            "qT": qTb,
            "kT": kTb,
            "vT": vTb,
            "wqT": np.ascontiguousarray((Wq[cs] * SCALE).T).astype(BF),
            "wkT": np.ascontiguousarray(Wk[cs].T).astype(BF),
            "wvT": np.ascontiguousarray(Wv[cs].T).astype(BF),
            "woT": np.ascontiguousarray(Wo[:, cs].T).astype(BF),
            "bqkv": bqkv,
        })
    return in_maps


def kernel(q, k, v, Wq, bq, Wk, bk, Wv, bv, Wo, bo, _want_results=False, **_unused):
    from concourse.bass_utils import run_bass_kernel_spmd

    q, k, v = (np.asarray(x, np.float32) for x in (q, k, v))
    Wq, bq, Wk, bk, Wv, bv, Wo, bo = (
        np.asarray(x, np.float32) for x in (Wq, bq, Wk, bk, Wv, bv, Wo, bo)
    )

    nc = _get_nc()
    in_maps = _make_in_maps(q, k, v, Wq, bq, Wk, bk, Wv, bv, Wo, bo)
    res = run_bass_kernel_spmd(nc, in_maps, core_ids=list(range(NCORES)))
    results = res.results

    out = np.zeros((B, S, E), np.float32)
    weights = np.empty((B, H, S, S), np.float32)
    for c in range(NCORES):
        b, g = divmod(c, HPG)
        out[b] += results[c]["o_out"].astype(np.float32)
        weights[b, g * HPG:(g + 1) * HPG] = results[c]["w_out"].astype(np.float32)
    out += bo
    if _want_results:
        return (out, weights), res
    return out, weights


# revision 18
# speedup vs baseline: 1.1066x; 1.1066x over previous
"""Multi-head attention (B=2, S=2048, E=1024, H=16) on 8 TRN2 NeuronCores.

Sharding: data-parallel over batch (2) x tensor-parallel over head groups (4).
Core c = b*4 + g handles batch b, heads [4g, 4g+4), channel slice [256g, 256g+256).

No collectives: each core computes its partial out-projection; the host sums the
4 head-group partials per batch and adds bo. Attention weights are written per
head directly in the reference [q, k] layout.

All matmuls use a full 128-row contraction: per-head tensors (dh=64) sit in
partitions 0-63 with partitions 64-127 zeroed ("z-layout"). Partial-array
matmuls (K=64) keep the PE clock throttled at 4/8 (HAM never registers enough
activity); zero-padding to K=128 keeps it at 8/8 for the whole kernel.

Device pipeline per core (bf16 compute, fp32 PSUM accumulation):
  phase 0: load pre-transposed inputs, project qp/kp (z-layout [head, token]),
           project vp and PE-transpose to token-major z-layout.
  per head:
    pass C: scores s[q,k] (q on partitions), exp on ScalarE with accum_out row
            sums -> Z, reciprocal, normalize, DMA weights out.
    pass A: scores st[k,q] (k on partitions), exp -> st bf16 blocks.
    pass B: AV matmuls -> U_T[dh,q] unnormalized.
  per pair: recip rows -> DRAM -> partition broadcast -> normalize U_T tile.
  tail:    out-projection through Wo shard, DMA partial out.
"""

import sys

for _p in ("/opt/trn_rl_repo",):
    if _p not in sys.path:
        sys.path.insert(0, _p)

import numpy as np
import ml_dtypes

B, S, E, H = 2, 2048, 1024, 16
DH = E // H
SCALE = DH ** -0.5
HPG = 4            # heads per group (per core)
CS = 256           # channel slice width per core
NCORES = 8
BF = ml_dtypes.bfloat16

_CACHE = {}


def _build_nc():
    import concourse.bass as bass
    import concourse.tile as tile
    from concourse import bacc, mybir
    from concourse.masks import make_identity

    DT = mybir.dt.bfloat16
    F32 = mybir.dt.float32
    Exp = mybir.ActivationFunctionType.Exp

    nc = bacc.Bacc(None)

    qT_ext = nc.declare_dram_parameter("qT", [E, S], DT, isOutput=False)
    kT_ext = nc.declare_dram_parameter("kT", [E, S], DT, isOutput=False)
    vT_ext = nc.declare_dram_parameter("vT", [E, S], DT, isOutput=False)
    wqT_ext = nc.declare_dram_parameter("wqT", [E, CS], DT, isOutput=False)
    wkT_ext = nc.declare_dram_parameter("wkT", [E, CS], DT, isOutput=False)
    wvT_ext = nc.declare_dram_parameter("wvT", [E, CS], DT, isOutput=False)
    woT_ext = nc.declare_dram_parameter("woT", [CS, E], DT, isOutput=False)
    bias_ext = nc.declare_dram_parameter("bias", [128, 16], F32, isOutput=False)

    w_out = nc.declare_dram_parameter("w_out", [HPG, S, S], DT, isOutput=True)
    o_out = nc.declare_dram_parameter("o_out", [S, E], DT, isOutput=True)
    recip_dram = nc.dram_tensor("recip_dram", [HPG * (S // 128), 128], DT)

    EC = E // 128      # 8 contraction chunks for projections
    QC = S // 512      # 4 token chunks of 512
    QT = S // 128      # 16 token tiles of 128
    KT = S // 128      # 16 key tiles of 128

    with tile.TileContext(nc) as tc:
        consts = tc.alloc_tile_pool(name="consts", bufs=1)
        acts = tc.alloc_tile_pool(name="acts", bufs=1)
        ps_big = tc.alloc_tile_pool(name="ps_big", bufs=2, space="PSUM")
        ps_av = tc.alloc_tile_pool(name="ps_av", bufs=2, space="PSUM")
        ps_t = tc.alloc_tile_pool(name="ps_t", bufs=1, space="PSUM")
        st_pool = tc.alloc_tile_pool(name="st", bufs=2)
        w_pool = tc.alloc_tile_pool(name="w", bufs=4)
        o_pool = tc.alloc_tile_pool(name="o", bufs=2)
        small = tc.alloc_tile_pool(name="small", bufs=4)

        ident = consts.tile([128, 128], DT)
        make_identity(nc, ident)
        bias_sb = consts.tile([128, 16], F32)
        nc.sync.dma_start(out=bias_sb, in_=bias_ext[:, :])
        wq_sb = consts.tile([128, EC, CS], DT)
        wk_sb = consts.tile([128, EC, CS], DT)
        wv_sb = consts.tile([128, EC, CS], DT)
        nc.sync.dma_start(out=wq_sb, in_=wqT_ext[:, :].rearrange("(c p) n -> p c n", p=128))
        nc.sync.dma_start(out=wk_sb, in_=wkT_ext[:, :].rearrange("(c p) n -> p c n", p=128))
        nc.sync.dma_start(out=wv_sb, in_=wvT_ext[:, :].rearrange("(c p) n -> p c n", p=128))
        woT_sb = consts.tile([128, 2, E], DT)
        nc.sync.dma_start(out=woT_sb, in_=woT_ext[:, :].rearrange("(c p) n -> p c n", p=128))

        # z-layout activations: head h in partitions 0-63, 64-127 zeroed
        qpT_sb = acts.tile([128, HPG, S], DT)
        kpT_sb = acts.tile([128, HPG, S], DT)
        vh_sb = acts.tile([128, KT, HPG, 128], DT)  # token-major v, cols 64-127 zero
        UT_sb = acts.tile([128, 2, S], DT)       # unnormalized attn out, transposed
        rb_sb = acts.tile([128, 2, S], DT)       # recip broadcast over partitions
        rcall_sb = acts.tile([128, HPG * (S // 128)], DT)  # recip columns

        nc.vector.memset(qpT_sb[64:128, :, :], 0.0)
        nc.vector.memset(kpT_sb[64:128, :, :], 0.0)
        nc.vector.memset(vh_sb[:, :, :, 64:128], 0.0)

        # ---- phase 0: projections ----
        xin = tc.alloc_tile_pool(name="xin", bufs=1)
        vtmp = tc.alloc_tile_pool(name="vtmp", bufs=1)
        if True:
            vpT_sb = vtmp.tile([128, 2, S], DT)
            for (x_ext, w_sb_t, zdst, bcol) in (
                (qT_ext, wq_sb, qpT_sb, 0),
                (kT_ext, wk_sb, kpT_sb, 4),
                (vT_ext, wv_sb, None, 8),
            ):
                x_sb = xin.tile([128, EC, S], DT, tag="xT")
                nc.sync.dma_start(out=x_sb, in_=x_ext[:, :].rearrange("(c p) t -> p c t", p=128))
                for ct in range(2):
                    for qcp in range(2):
                        ps = ps_big.tile([128, 2, 512], F32, tag="ps_big")
                        for j in range(2):
                            qc = qcp * 2 + j
                            for ec in range(EC):
                                nc.tensor.matmul(
                                    ps[:, j, :],
                                    lhsT=w_sb_t[:, ec, ct * 128:(ct + 1) * 128],
                                    rhs=x_sb[:, ec, qc * 512:(qc + 1) * 512],
                                    start=(ec == 0),
                                    stop=(ec == EC - 1),
                                )
                        for j in range(2):
                            qc = qcp * 2 + j
                            sl = slice(qc * 512, (qc + 1) * 512)
                            if zdst is not None:
                                for hh in range(2):
                                    h = 2 * ct + hh
                                    nc.vector.tensor_scalar_add(
                                        out=zdst[0:64, h, sl],
                                        in0=ps[hh * 64:(hh + 1) * 64, j, :],
                                        scalar1=bias_sb[0:64, bcol + h:bcol + h + 1],
                                    )
                            else:
                                nc.vector.tensor_scalar_add(
                                    out=vpT_sb[:, ct, sl],
                                    in0=ps[:, j, :],
                                    scalar1=bias_sb[:, 12 + ct:12 + ct + 1],
                                )
            # transpose vp: [channel, token] -> token-major z-layout head slices
            for ct in range(2):
                for tt in range(KT):
                    pst = ps_t.tile([128, 128], DT, tag="ps_t")
                    nc.tensor.transpose(pst, vpT_sb[:, ct, tt * 128:(tt + 1) * 128], ident)
                    for hh in range(2):
                        h = 2 * ct + hh
                        nc.vector.tensor_copy(
                            out=vh_sb[:, tt, h, 0:64],
                            in_=pst[:, hh * 64:(hh + 1) * 64],
                        )
        vtmp.release()
        xin.release()

        # ---- per-head attention, recip/normalize per head pair ----
        for hp in range(2):
            h0, h1 = 2 * hp, 2 * hp + 1
            for h in (h0, h1):
                # pass C: s[q, k] + softmax weights output
                for qt in range(QT):
                    ps = ps_big.tile([128, 2, 512], F32, tag="ps_big")
                    ps2 = ps_big.tile([128, 2, 512], F32, tag="ps_big")
                    w_sb = w_pool.tile([128, 4, 512], DT, tag="w")
                    zparts = small.tile([128, 2], F32, tag="zp")
                    for half, psh in ((0, ps), (1, ps2)):
                        for kc2 in range(2):
                            kc = half * 2 + kc2
                            nc.tensor.matmul(
                                psh[:, kc2, :],
                                lhsT=qpT_sb[:, h, qt * 128:(qt + 1) * 128],
                                rhs=kpT_sb[:, h, kc * 512:(kc + 1) * 512],
                                start=True,
                                stop=True,
                            )
                        nc.scalar.activation(
                            out=w_sb[:, half * 2:half * 2 + 2, :],
                            in_=psh[:, :, :],
                            func=Exp,
                            accum_out=zparts[:, half:half + 1],
                        )
                    z = small.tile([128, 1], F32, tag="z")
                    nc.vector.tensor_add(z, zparts[:, 0:1], zparts[:, 1:2])
                    rc = small.tile([128, 1], F32, tag="rc")
                    nc.vector.reciprocal(rc, z)
                    nc.vector.tensor_scalar_mul(out=w_sb, in0=w_sb, scalar1=rc)
                    nc.sync.dma_start(
                        out=w_out[h, qt * 128:(qt + 1) * 128, :],
                        in_=w_sb[:, :, :],
                    )
                    nc.vector.tensor_copy(
                        out=rcall_sb[:, h * QT + qt:h * QT + qt + 1], in_=rc,
                    )

                # pass A/B: st[k, q] blocks + AV
                for qc in range(QC):
                    st_blk = st_pool.tile([128, KT, 512], DT, tag="st")
                    for ktp in range(KT // 2):
                        ps = ps_big.tile([128, 2, 512], F32, tag="ps_big")
                        for j in range(2):
                            kt = ktp * 2 + j
                            nc.tensor.matmul(
                                ps[:, j, :],
                                lhsT=kpT_sb[:, h, kt * 128:(kt + 1) * 128],
                                rhs=qpT_sb[:, h, qc * 512:(qc + 1) * 512],
                                start=True,
                                stop=True,
                            )
                        nc.scalar.activation(
                            out=st_blk[:, ktp * 2:ktp * 2 + 2, :],
                            in_=ps[:, :, :],
                            func=Exp,
                        )
                    av = ps_av.tile([128, 512], F32, tag="av")
                    for kt in range(KT):
                        nc.tensor.matmul(
                            av,
                            lhsT=vh_sb[:, kt, h, :],
                            rhs=st_blk[:, kt, :],
                            start=(kt == 0),
                            stop=(kt == KT - 1),
                        )
                    po = (h % 2) * 64
                    nc.vector.tensor_copy(
                        out=UT_sb[po:po + 64, hp, qc * 512:(qc + 1) * 512],
                        in_=av[0:64, :],
                    )

            # recip rows for this pair -> DRAM -> partition broadcast -> normalize UT
            pst = ps_t.tile([32, 128], DT, tag="ps_t2")
            nc.tensor.transpose(pst, rcall_sb[:, hp * 2 * QT:(hp + 1) * 2 * QT], ident)
            rrow_sb = small.tile([32, 128], DT, tag="rrow")
            nc.vector.tensor_copy(out=rrow_sb, in_=pst)
            nc.sync.dma_start(
                out=recip_dram[hp * 2 * QT:(hp + 1) * 2 * QT, :], in_=rrow_sb[:, :],
            )
            for hh in range(2):
                h = 2 * hp + hh
                nc.gpsimd.dma_start(
                    out=rb_sb[hh * 64:hh * 64 + 64, hp, :],
                    in_=recip_dram[h * QT:(h + 1) * QT, :].rearrange("a b -> (a b)")[None, :].to_broadcast([64, S]),
                )
            nc.vector.tensor_mul(
                out=UT_sb[:, hp, :],
                in0=UT_sb[:, hp, :],
                in1=rb_sb[:, hp, :],
            )

        # ---- tail: out projection ----
        for qt in range(QT):
            ps = ps_big.tile([128, 2, 512], F32, tag="ps_big")
            for ec in range(2):
                for cc in range(2):
                    nc.tensor.matmul(
                        ps[:, ec, :],
                        lhsT=UT_sb[:, cc, qt * 128:(qt + 1) * 128],
                        rhs=woT_sb[:, cc, ec * 512:(ec + 1) * 512],
                        start=(cc == 0),
                        stop=(cc == 1),
                    )
            o_sb = o_pool.tile([128, 2, 512], DT, tag="o")
            nc.vector.tensor_copy(out=o_sb, in_=ps)
            nc.sync.dma_start(out=o_out[qt * 128:(qt + 1) * 128, :], in_=o_sb[:, :, :])

        for p in (small, o_pool, w_pool, st_pool, ps_t, ps_av, ps_big, acts, consts):
            p.release()

    nc.finalize()
    return nc


def _get_nc():
    if "nc" not in _CACHE:
        _CACHE["nc"] = _build_nc()
    return _CACHE["nc"]


def _make_in_maps(q, k, v, Wq, bq, Wk, bk, Wv, bv, Wo, bo):
    in_maps = []
    perb = []
    for b in range(B):
        perb.append((
            np.ascontiguousarray(q[b].T).astype(BF),
            np.ascontiguousarray(k[b].T).astype(BF),
            np.ascontiguousarray(v[b].T).astype(BF),
        ))
    for c in range(NCORES):
        b, g = divmod(c, HPG)
        cs = slice(g * CS, (g + 1) * CS)
        qTb, kTb, vTb = perb[b]
        bias = np.zeros((128, 16), np.float32)
        for col0, vec in ((0, bq[cs] * SCALE), (4, bk[cs])):
            for h in range(HPG):
                bias[:64, col0 + h] = vec[h * 64:(h + 1) * 64]
        bias[:, 12] = bv[cs][:128]
        bias[:, 13] = bv[cs][128:]
        in_maps.append({
            "qT": qTb,
            "kT": kTb,
            "vT": vTb,
            "wqT": np.ascontiguousarray((Wq[cs] * SCALE).T).astype(BF),
            "wkT": np.ascontiguousarray(Wk[cs].T).astype(BF),
            "wvT": np.ascontiguousarray(Wv[cs].T).astype(BF),
            "woT": np.ascontiguousarray(Wo[:, cs].T).astype(BF),
            "bias": bias,
        })
    return in_maps


def kernel(q, k, v, Wq, bq, Wk, bk, Wv, bv, Wo, bo, _want_results=False, **_unused):
    from concourse.bass_utils import run_bass_kernel_spmd

    q, k, v = (np.asarray(x, np.float32) for x in (q, k, v))
    Wq, bq, Wk, bk, Wv, bv, Wo, bo = (
        np.asarray(x, np.float32) for x in (Wq, bq, Wk, bk, Wv, bv, Wo, bo)
    )

    nc = _get_nc()
    in_maps = _make_in_maps(q, k, v, Wq, bq, Wk, bk, Wv, bv, Wo, bo)
    res = run_bass_kernel_spmd(nc, in_maps, core_ids=list(range(NCORES)))
    results = res.results

    out = np.zeros((B, S, E), np.float32)
    weights = np.empty((B, H, S, S), np.float32)
    for c in range(NCORES):
        b, g = divmod(c, HPG)
        out[b] += results[c]["o_out"].astype(np.float32)
        weights[b, g * HPG:(g + 1) * HPG] = results[c]["w_out"].astype(np.float32)
    out += bo
    if _want_results:
        return (out, weights), res
    return out, weights


# revision 19
# speedup vs baseline: 1.2049x; 1.0888x over previous
"""Multi-head attention (B=2, S=2048, E=1024, H=16) on 8 TRN2 NeuronCores.

Sharding: data-parallel over batch (2) x tensor-parallel over head groups (4).
Core c = b*4 + g handles batch b, heads [4g, 4g+4), channel slice [256g, 256g+256).

No collectives: each core computes its partial out-projection; the host sums the
4 head-group partials per batch and adds bo. Attention weights are written per
head directly in the reference [q, k] layout.

All matmuls use a full 128-row contraction: per-head tensors (dh=64) sit in
partitions 0-63 with partitions 64-127 zeroed ("z-layout"). Partial-array
matmuls (K=64) keep the PE clock throttled at 4/8 (HAM never registers enough
activity); zero-padding to K=128 keeps it at 8/8 for the whole kernel.

Device pipeline per core (bf16 compute, fp32 PSUM accumulation):
  phase 0: load pre-transposed inputs, project qp/kp (z-layout [head, token]),
           project vp and PE-transpose to token-major z-layout.
  per head:
    pass C: scores s[q,k] (q on partitions), exp on ScalarE with accum_out row
            sums -> Z, reciprocal, normalize, DMA weights out.
    pass A: scores st[k,q] (k on partitions), exp -> st bf16 blocks.
    pass B: AV matmuls -> U_T[dh,q] unnormalized.
  per pair: recip rows -> DRAM -> partition broadcast -> normalize U_T tile.
  tail:    out-projection through Wo shard, DMA partial out.
"""

import sys

for _p in ("/opt/trn_rl_repo",):
    if _p not in sys.path:
        sys.path.insert(0, _p)

import numpy as np
import ml_dtypes

B, S, E, H = 2, 2048, 1024, 16
DH = E // H
SCALE = DH ** -0.5
HPG = 4            # heads per group (per core)
CS = 256           # channel slice width per core
NCORES = 8
BF = ml_dtypes.bfloat16

_CACHE = {}


def _build_nc():
    import concourse.bass as bass
    import concourse.tile as tile
    from concourse import bacc, mybir
    from concourse.masks import make_identity

    DT = mybir.dt.bfloat16
    F32 = mybir.dt.float32
    Exp = mybir.ActivationFunctionType.Exp

    nc = bacc.Bacc(None)

    qT_ext = nc.declare_dram_parameter("qT", [E, S], DT, isOutput=False)
    kT_ext = nc.declare_dram_parameter("kT", [E, S], DT, isOutput=False)
    vT_ext = nc.declare_dram_parameter("vT", [E, S], DT, isOutput=False)
    wqT_ext = nc.declare_dram_parameter("wqT", [E, CS], DT, isOutput=False)
    wkT_ext = nc.declare_dram_parameter("wkT", [E, CS], DT, isOutput=False)
    wvT_ext = nc.declare_dram_parameter("wvT", [E, CS], DT, isOutput=False)
    woT_ext = nc.declare_dram_parameter("woT", [CS, E], DT, isOutput=False)
    bias_ext = nc.declare_dram_parameter("bias", [128, 16], F32, isOutput=False)

    w_out = nc.declare_dram_parameter("w_out", [HPG, S, S], DT, isOutput=True)
    o_out = nc.declare_dram_parameter("o_out", [S, E], DT, isOutput=True)
    recip_dram = nc.dram_tensor("recip_dram", [HPG * (S // 128), 128], DT)

    EC = E // 128      # 8 contraction chunks for projections
    QC = S // 512      # 4 token chunks of 512
    QT = S // 128      # 16 token tiles of 128
    KT = S // 128      # 16 key tiles of 128

    with tile.TileContext(nc) as tc:
        consts = tc.alloc_tile_pool(name="consts", bufs=1)
        acts = tc.alloc_tile_pool(name="acts", bufs=1)
        ps_big = tc.alloc_tile_pool(name="ps_big", bufs=2, space="PSUM")
        ps_av = tc.alloc_tile_pool(name="ps_av", bufs=2, space="PSUM")
        ps_t = tc.alloc_tile_pool(name="ps_t", bufs=1, space="PSUM")
        st_pool = tc.alloc_tile_pool(name="st", bufs=2)
        w_pool = tc.alloc_tile_pool(name="w", bufs=4)
        o_pool = tc.alloc_tile_pool(name="o", bufs=2)
        small = tc.alloc_tile_pool(name="small", bufs=4)

        ident = consts.tile([128, 128], DT)
        make_identity(nc, ident)
        bias_sb = consts.tile([128, 16], F32)
        nc.sync.dma_start(out=bias_sb, in_=bias_ext[:, :])
        wq_sb = consts.tile([128, EC, CS], DT)
        wk_sb = consts.tile([128, EC, CS], DT)
        wv_sb = consts.tile([128, EC, CS], DT)
        nc.sync.dma_start(out=wq_sb, in_=wqT_ext[:, :].rearrange("(c p) n -> p c n", p=128))
        nc.sync.dma_start(out=wk_sb, in_=wkT_ext[:, :].rearrange("(c p) n -> p c n", p=128))
        nc.sync.dma_start(out=wv_sb, in_=wvT_ext[:, :].rearrange("(c p) n -> p c n", p=128))
        woT_sb = consts.tile([128, 2, E], DT)
        nc.sync.dma_start(out=woT_sb, in_=woT_ext[:, :].rearrange("(c p) n -> p c n", p=128))

        # z-layout activations: head h in partitions 0-63, 64-127 zeroed
        qpT_sb = acts.tile([128, HPG, S], DT)
        kpT_sb = acts.tile([128, HPG, S], DT)
        vh_sb = acts.tile([128, KT, HPG, 128], DT)  # token-major v, cols 64-127 zero
        UT_sb = acts.tile([128, 2, S], DT)       # unnormalized attn out, transposed
        rb_sb = acts.tile([128, 2, S], DT)       # recip broadcast over partitions
        rcall_sb = acts.tile([128, HPG * (S // 128)], DT)  # recip columns

        nc.gpsimd.memset(qpT_sb[64:128, :, :], 0.0)
        nc.gpsimd.memset(kpT_sb[64:128, :, :], 0.0)
        nc.gpsimd.memset(vh_sb[:, :, :, 64:128], 0.0)

        # ---- phase 0: projections ----
        xin = tc.alloc_tile_pool(name="xin", bufs=2)
        vtmp = tc.alloc_tile_pool(name="vtmp", bufs=1)
        if True:
            vpT_sb = vtmp.tile([128, 2, S], DT)
            for (x_ext, w_sb_t, zdst, bcol) in (
                (qT_ext, wq_sb, qpT_sb, 0),
                (kT_ext, wk_sb, kpT_sb, 4),
                (vT_ext, wv_sb, None, 8),
            ):
                for half in range(2):
                    x_sb = xin.tile([128, EC, S // 2], DT, tag="xT")
                    nc.sync.dma_start(
                        out=x_sb,
                        in_=x_ext[:, half * 1024:(half + 1) * 1024].rearrange("(c p) t -> p c t", p=128),
                    )
                    for ct in range(2):
                        qcp = half
                        ps = ps_big.tile([128, 2, 512], F32, tag="ps_big")
                        for j in range(2):
                            qc = qcp * 2 + j
                            for ec in range(EC):
                                nc.tensor.matmul(
                                    ps[:, j, :],
                                    lhsT=w_sb_t[:, ec, ct * 128:(ct + 1) * 128],
                                    rhs=x_sb[:, ec, j * 512:(j + 1) * 512],
                                    start=(ec == 0),
                                    stop=(ec == EC - 1),
                                )
                        for j in range(2):
                            qc = qcp * 2 + j
                            sl = slice(qc * 512, (qc + 1) * 512)
                            if zdst is not None:
                                for hh in range(2):
                                    h = 2 * ct + hh
                                    nc.vector.tensor_scalar_add(
                                        out=zdst[0:64, h, sl],
                                        in0=ps[hh * 64:(hh + 1) * 64, j, :],
                                        scalar1=bias_sb[0:64, bcol + h:bcol + h + 1],
                                    )
                            else:
                                nc.vector.tensor_scalar_add(
                                    out=vpT_sb[:, ct, sl],
                                    in0=ps[:, j, :],
                                    scalar1=bias_sb[:, 12 + ct:12 + ct + 1],
                                )
            # transpose vp: [channel, token] -> token-major z-layout head slices
            for ct in range(2):
                for tt in range(KT):
                    pst = ps_t.tile([128, 128], DT, tag="ps_t")
                    nc.tensor.transpose(pst, vpT_sb[:, ct, tt * 128:(tt + 1) * 128], ident)
                    for hh in range(2):
                        h = 2 * ct + hh
                        nc.vector.tensor_copy(
                            out=vh_sb[:, tt, h, 0:64],
                            in_=pst[:, hh * 64:(hh + 1) * 64],
                        )
        vtmp.release()
        xin.release()

        # ---- per-head attention, recip/normalize per head pair ----
        for hp in range(2):
            h0, h1 = 2 * hp, 2 * hp + 1
            for h in (h0, h1):
                # pass C: s[q, k] + softmax weights output
                for qt in range(QT):
                    ps = ps_big.tile([128, 2, 512], F32, tag="ps_big")
                    ps2 = ps_big.tile([128, 2, 512], F32, tag="ps_big")
                    w_sb = w_pool.tile([128, 4, 512], DT, tag="w")
                    zparts = small.tile([128, 2], F32, tag="zp")
                    for half, psh in ((0, ps), (1, ps2)):
                        for kc2 in range(2):
                            kc = half * 2 + kc2
                            nc.tensor.matmul(
                                psh[:, kc2, :],
                                lhsT=qpT_sb[:, h, qt * 128:(qt + 1) * 128],
                                rhs=kpT_sb[:, h, kc * 512:(kc + 1) * 512],
                                start=True,
                                stop=True,
                            )
                        nc.scalar.activation(
                            out=w_sb[:, half * 2:half * 2 + 2, :],
                            in_=psh[:, :, :],
                            func=Exp,
                            accum_out=zparts[:, half:half + 1],
                        )
                    z = small.tile([128, 1], F32, tag="z")
                    nc.vector.tensor_add(z, zparts[:, 0:1], zparts[:, 1:2])
                    rc = small.tile([128, 1], F32, tag="rc")
                    nc.vector.reciprocal(rc, z)
                    nc.vector.tensor_scalar_mul(out=w_sb, in0=w_sb, scalar1=rc)
                    nc.sync.dma_start(
                        out=w_out[h, qt * 128:(qt + 1) * 128, :],
                        in_=w_sb[:, :, :],
                    )
                    nc.vector.tensor_copy(
                        out=rcall_sb[:, h * QT + qt:h * QT + qt + 1], in_=rc,
                    )

                # pass A/B: st[k, q] blocks + AV
                for qc in range(QC):
                    st_blk = st_pool.tile([128, KT, 512], DT, tag="st")
                    for ktp in range(KT // 2):
                        ps = ps_big.tile([128, 2, 512], F32, tag="ps_big")
                        for j in range(2):
                            kt = ktp * 2 + j
                            nc.tensor.matmul(
                                ps[:, j, :],
                                lhsT=kpT_sb[:, h, kt * 128:(kt + 1) * 128],
                                rhs=qpT_sb[:, h, qc * 512:(qc + 1) * 512],
                                start=True,
                                stop=True,
                            )
                        nc.scalar.activation(
                            out=st_blk[:, ktp * 2:ktp * 2 + 2, :],
                            in_=ps[:, :, :],
                            func=Exp,
                        )
                    av = ps_av.tile([128, 512], F32, tag="av")
                    for kt in range(KT):
                        nc.tensor.matmul(
                            av,
                            lhsT=vh_sb[:, kt, h, :],
                            rhs=st_blk[:, kt, :],
                            start=(kt == 0),
                            stop=(kt == KT - 1),
                        )
                    po = (h % 2) * 64
                    nc.vector.tensor_copy(
                        out=UT_sb[po:po + 64, hp, qc * 512:(qc + 1) * 512],
                        in_=av[0:64, :],
                    )

            # recip rows for this pair -> DRAM -> partition broadcast -> normalize UT
            pst = ps_t.tile([32, 128], DT, tag="ps_t2")
            nc.tensor.transpose(pst, rcall_sb[:, hp * 2 * QT:(hp + 1) * 2 * QT], ident)
            rrow_sb = small.tile([32, 128], DT, tag="rrow")
            nc.vector.tensor_copy(out=rrow_sb, in_=pst)
            nc.sync.dma_start(
                out=recip_dram[hp * 2 * QT:(hp + 1) * 2 * QT, :], in_=rrow_sb[:, :],
            )
            for hh in range(2):
                h = 2 * hp + hh
                nc.gpsimd.dma_start(
                    out=rb_sb[hh * 64:hh * 64 + 64, hp, :],
                    in_=recip_dram[h * QT:(h + 1) * QT, :].rearrange("a b -> (a b)")[None, :].to_broadcast([64, S]),
                )
            nc.vector.tensor_mul(
                out=UT_sb[:, hp, :],
                in0=UT_sb[:, hp, :],
                in1=rb_sb[:, hp, :],
            )

        # ---- tail: out projection ----
        for qt in range(QT):
            ps = ps_big.tile([128, 2, 512], F32, tag="ps_big")
            for ec in range(2):
                for cc in range(2):
                    nc.tensor.matmul(
                        ps[:, ec, :],
                        lhsT=UT_sb[:, cc, qt * 128:(qt + 1) * 128],
                        rhs=woT_sb[:, cc, ec * 512:(ec + 1) * 512],
                        start=(cc == 0),
                        stop=(cc == 1),
                    )
            o_sb = o_pool.tile([128, 2, 512], DT, tag="o")
            nc.vector.tensor_copy(out=o_sb, in_=ps)
            nc.sync.dma_start(out=o_out[qt * 128:(qt + 1) * 128, :], in_=o_sb[:, :, :])

        for p in (small, o_pool, w_pool, st_pool, ps_t, ps_av, ps_big, acts, consts):
            p.release()

    nc.finalize()
    return nc


def _get_nc():
    if "nc" not in _CACHE:
        _CACHE["nc"] = _build_nc()
    return _CACHE["nc"]


def _make_in_maps(q, k, v, Wq, bq, Wk, bk, Wv, bv, Wo, bo):
    in_maps = []
    perb = []
    for b in range(B):
        perb.append((
            np.ascontiguousarray(q[b].T).astype(BF),
            np.ascontiguousarray(k[b].T).astype(BF),
            np.ascontiguousarray(v[b].T).astype(BF),
        ))
    for c in range(NCORES):
        b, g = divmod(c, HPG)
        cs = slice(g * CS, (g + 1) * CS)
        qTb, kTb, vTb = perb[b]
        bias = np.zeros((128, 16), np.float32)
        for col0, vec in ((0, bq[cs] * SCALE), (4, bk[cs])):
            for h in range(HPG):
                bias[:64, col0 + h] = vec[h * 64:(h + 1) * 64]
        bias[:, 12] = bv[cs][:128]
        bias[:, 13] = bv[cs][128:]
        in_maps.append({
            "qT": qTb,
            "kT": kTb,
            "vT": vTb,
            "wqT": np.ascontiguousarray((Wq[cs] * SCALE).T).astype(BF),
            "wkT": np.ascontiguousarray(Wk[cs].T).astype(BF),
            "wvT": np.ascontiguousarray(Wv[cs].T).astype(BF),
            "woT": np.ascontiguousarray(Wo[:, cs].T).astype(BF),
            "bias": bias,
        })
    return in_maps


def kernel(q, k, v, Wq, bq, Wk, bk, Wv, bv, Wo, bo, _want_results=False, **_unused):
    from concourse.bass_utils import run_bass_kernel_spmd

    q, k, v = (np.asarray(x, np.float32) for x in (q, k, v))
    Wq, bq, Wk, bk, Wv, bv, Wo, bo = (
        np.asarray(x, np.float32) for x in (Wq, bq, Wk, bk, Wv, bv, Wo, bo)
    )

    nc = _get_nc()
    in_maps = _make_in_maps(q, k, v, Wq, bq, Wk, bk, Wv, bv, Wo, bo)
    res = run_bass_kernel_spmd(nc, in_maps, core_ids=list(range(NCORES)))
    results = res.results

    out = np.zeros((B, S, E), np.float32)
    weights = np.empty((B, H, S, S), np.float32)
    for c in range(NCORES):
        b, g = divmod(c, HPG)
        out[b] += results[c]["o_out"].astype(np.float32)
        weights[b, g * HPG:(g + 1) * HPG] = results[c]["w_out"].astype(np.float32)
    out += bo
    if _want_results:
        return (out, weights), res
    return out, weights


# revision 20
# speedup vs baseline: 1.2096x; 1.0039x over previous
"""Multi-head attention (B=2, S=2048, E=1024, H=16) on 8 TRN2 NeuronCores.

Sharding: data-parallel over batch (2) x tensor-parallel over head groups (4).
Core c = b*4 + g handles batch b, heads [4g, 4g+4), channel slice [256g, 256g+256).

No collectives: each core computes its partial out-projection; the host sums the
4 head-group partials per batch and adds bo. Attention weights are written per
head directly in the reference [q, k] layout.

All matmuls use a full 128-row contraction: per-head tensors (dh=64) sit in
partitions 0-63 with partitions 64-127 zeroed ("z-layout"). Partial-array
matmuls (K=64) keep the PE clock throttled at 4/8 (HAM never registers enough
activity); zero-padding to K=128 keeps it at 8/8 for the whole kernel.

Device pipeline per core (bf16 compute, fp32 PSUM accumulation):
  phase 0: load pre-transposed inputs, project qp/kp (z-layout [head, token]),
           project vp and PE-transpose to token-major z-layout.
  per head:
    pass C: scores s[q,k] (q on partitions), exp on ScalarE with accum_out row
            sums -> Z, reciprocal, normalize, DMA weights out.
    pass A: scores st[k,q] (k on partitions), exp -> st bf16 blocks.
    pass B: AV matmuls -> U_T[dh,q] unnormalized.
  per pair: recip rows -> DRAM -> partition broadcast -> normalize U_T tile.
  tail:    out-projection through Wo shard, DMA partial out.
"""

import sys

for _p in ("/opt/trn_rl_repo",):
    if _p not in sys.path:
        sys.path.insert(0, _p)

import numpy as np
import ml_dtypes

B, S, E, H = 2, 2048, 1024, 16
DH = E // H
SCALE = DH ** -0.5
HPG = 4            # heads per group (per core)
CS = 256           # channel slice width per core
NCORES = 8
BF = ml_dtypes.bfloat16

_CACHE = {}


def _build_nc():
    import concourse.bass as bass
    import concourse.tile as tile
    from concourse import bacc, mybir
    from concourse.masks import make_identity

    DT = mybir.dt.bfloat16
    F32 = mybir.dt.float32
    Exp = mybir.ActivationFunctionType.Exp

    nc = bacc.Bacc(None)

    qT_ext = nc.declare_dram_parameter("qT", [E, S], DT, isOutput=False)
    kT_ext = nc.declare_dram_parameter("kT", [E, S], DT, isOutput=False)
    vT_ext = nc.declare_dram_parameter("vT", [E, S], DT, isOutput=False)
    wqT_ext = nc.declare_dram_parameter("wqT", [E, CS], DT, isOutput=False)
    wkT_ext = nc.declare_dram_parameter("wkT", [E, CS], DT, isOutput=False)
    wvT_ext = nc.declare_dram_parameter("wvT", [E, CS], DT, isOutput=False)
    woT_ext = nc.declare_dram_parameter("woT", [CS, E], DT, isOutput=False)
    bias_ext = nc.declare_dram_parameter("bias", [128, 16], F32, isOutput=False)

    w_out = nc.declare_dram_parameter("w_out", [HPG, S, S], DT, isOutput=True)
    o_out = nc.declare_dram_parameter("o_out", [S, E], DT, isOutput=True)
    recip_dram = nc.dram_tensor("recip_dram", [HPG * (S // 128), 128], DT)

    EC = E // 128      # 8 contraction chunks for projections
    QC = S // 512      # 4 token chunks of 512
    QT = S // 128      # 16 token tiles of 128
    KT = S // 128      # 16 key tiles of 128

    with tile.TileContext(nc) as tc:
        consts = tc.alloc_tile_pool(name="consts", bufs=1)
        acts = tc.alloc_tile_pool(name="acts", bufs=1)
        ps_big = tc.alloc_tile_pool(name="ps_big", bufs=3, space="PSUM")
        st_pool = tc.alloc_tile_pool(name="st", bufs=2)
        w_pool = tc.alloc_tile_pool(name="w", bufs=4)
        o_pool = tc.alloc_tile_pool(name="o", bufs=2)
        small = tc.alloc_tile_pool(name="small", bufs=4)

        ident = consts.tile([128, 128], DT)
        make_identity(nc, ident)
        bias_sb = consts.tile([128, 16], F32)
        nc.sync.dma_start(out=bias_sb, in_=bias_ext[:, :])
        wq_sb = consts.tile([128, EC, CS], DT)
        wk_sb = consts.tile([128, EC, CS], DT)
        wv_sb = consts.tile([128, EC, CS], DT)
        nc.sync.dma_start(out=wq_sb, in_=wqT_ext[:, :].rearrange("(c p) n -> p c n", p=128))
        nc.sync.dma_start(out=wk_sb, in_=wkT_ext[:, :].rearrange("(c p) n -> p c n", p=128))
        nc.sync.dma_start(out=wv_sb, in_=wvT_ext[:, :].rearrange("(c p) n -> p c n", p=128))
        woT_sb = consts.tile([128, 2, E], DT)
        nc.sync.dma_start(out=woT_sb, in_=woT_ext[:, :].rearrange("(c p) n -> p c n", p=128))

        # z-layout activations: head h in partitions 0-63, 64-127 zeroed
        qpT_sb = acts.tile([128, HPG, S], DT)
        kpT_sb = acts.tile([128, HPG, S], DT)
        vh_sb = acts.tile([128, KT, HPG, 128], DT)  # token-major v, cols 64-127 zero
        UT_sb = acts.tile([128, 2, S], DT)       # unnormalized attn out, transposed
        rb_sb = acts.tile([128, 2, S], DT)       # recip broadcast over partitions
        rcall_sb = acts.tile([128, HPG * (S // 128)], DT)  # recip columns

        nc.gpsimd.memset(qpT_sb[64:128, :, :], 0.0)
        nc.gpsimd.memset(kpT_sb[64:128, :, :], 0.0)
        nc.gpsimd.memset(vh_sb[:, :, :, 64:128], 0.0)

        # ---- phase 0: projections ----
        xin = tc.alloc_tile_pool(name="xin", bufs=2)
        vtmp = tc.alloc_tile_pool(name="vtmp", bufs=1)
        ps_t = tc.alloc_tile_pool(name="ps_t", bufs=1, space="PSUM")
        if True:
            vpT_sb = vtmp.tile([128, 2, S], DT)
            for (x_ext, w_sb_t, zdst, bcol) in (
                (qT_ext, wq_sb, qpT_sb, 0),
                (kT_ext, wk_sb, kpT_sb, 4),
                (vT_ext, wv_sb, None, 8),
            ):
                for half in range(2):
                    x_sb = xin.tile([128, EC, S // 2], DT, tag="xT")
                    nc.sync.dma_start(
                        out=x_sb,
                        in_=x_ext[:, half * 1024:(half + 1) * 1024].rearrange("(c p) t -> p c t", p=128),
                    )
                    for ct in range(2):
                        qcp = half
                        ps = ps_big.tile([128, 2, 512], F32, tag="ps_big")
                        for j in range(2):
                            qc = qcp * 2 + j
                            for ec in range(EC):
                                nc.tensor.matmul(
                                    ps[:, j, :],
                                    lhsT=w_sb_t[:, ec, ct * 128:(ct + 1) * 128],
                                    rhs=x_sb[:, ec, j * 512:(j + 1) * 512],
                                    start=(ec == 0),
                                    stop=(ec == EC - 1),
                                )
                        for j in range(2):
                            qc = qcp * 2 + j
                            sl = slice(qc * 512, (qc + 1) * 512)
                            if zdst is not None:
                                for hh in range(2):
                                    h = 2 * ct + hh
                                    nc.vector.tensor_scalar_add(
                                        out=zdst[0:64, h, sl],
                                        in0=ps[hh * 64:(hh + 1) * 64, j, :],
                                        scalar1=bias_sb[0:64, bcol + h:bcol + h + 1],
                                    )
                            else:
                                nc.vector.tensor_scalar_add(
                                    out=vpT_sb[:, ct, sl],
                                    in0=ps[:, j, :],
                                    scalar1=bias_sb[:, 12 + ct:12 + ct + 1],
                                )
            # transpose vp: [channel, token] -> token-major z-layout head slices
            for ct in range(2):
                for tt in range(KT):
                    pst = ps_t.tile([128, 128], DT, tag="ps_t")
                    nc.tensor.transpose(pst, vpT_sb[:, ct, tt * 128:(tt + 1) * 128], ident)
                    for hh in range(2):
                        h = 2 * ct + hh
                        nc.vector.tensor_copy(
                            out=vh_sb[:, tt, h, 0:64],
                            in_=pst[:, hh * 64:(hh + 1) * 64],
                        )
        ps_t.release()
        vtmp.release()
        xin.release()

        ps_av = tc.alloc_tile_pool(name="ps_av", bufs=2, space="PSUM")

        # ---- per-head attention, recip/normalize per head pair ----
        for h in range(HPG):
            hp = h // 2
            if True:
                # pass C: s[q, k] + softmax weights output
                for qt in range(QT):
                    ps = ps_big.tile([128, 2, 512], F32, tag="ps_big")
                    ps2 = ps_big.tile([128, 2, 512], F32, tag="ps_big")
                    w_sb = w_pool.tile([128, 4, 512], DT, tag="w")
                    zparts = small.tile([128, 2], F32, tag="zp")
                    for half, psh in ((0, ps), (1, ps2)):
                        for kc2 in range(2):
                            kc = half * 2 + kc2
                            nc.tensor.matmul(
                                psh[:, kc2, :],
                                lhsT=qpT_sb[:, h, qt * 128:(qt + 1) * 128],
                                rhs=kpT_sb[:, h, kc * 512:(kc + 1) * 512],
                                start=True,
                                stop=True,
                            )
                        nc.scalar.activation(
                            out=w_sb[:, half * 2:half * 2 + 2, :],
                            in_=psh[:, :, :],
                            func=Exp,
                            accum_out=zparts[:, half:half + 1],
                        )
                    z = small.tile([128, 1], F32, tag="z")
                    nc.vector.tensor_add(z, zparts[:, 0:1], zparts[:, 1:2])
                    rc = small.tile([128, 1], F32, tag="rc")
                    nc.vector.reciprocal(rc, z)
                    nc.vector.tensor_scalar_mul(out=w_sb, in0=w_sb, scalar1=rc)
                    nc.sync.dma_start(
                        out=w_out[h, qt * 128:(qt + 1) * 128, :],
                        in_=w_sb[:, :, :],
                    )
                    nc.vector.tensor_copy(
                        out=rcall_sb[:, h * QT + qt:h * QT + qt + 1], in_=rc,
                    )

                # pass A/B: st[k, q] blocks + AV
                for qc in range(QC):
                    st_blk = st_pool.tile([128, KT, 512], DT, tag="st")
                    for ktp in range(KT // 2):
                        ps = ps_big.tile([128, 2, 512], F32, tag="ps_big")
                        for j in range(2):
                            kt = ktp * 2 + j
                            nc.tensor.matmul(
                                ps[:, j, :],
                                lhsT=kpT_sb[:, h, kt * 128:(kt + 1) * 128],
                                rhs=qpT_sb[:, h, qc * 512:(qc + 1) * 512],
                                start=True,
                                stop=True,
                            )
                        nc.scalar.activation(
                            out=st_blk[:, ktp * 2:ktp * 2 + 2, :],
                            in_=ps[:, :, :],
                            func=Exp,
                        )
                    av = ps_av.tile([128, 512], F32, tag="av")
                    for kt in range(KT):
                        nc.tensor.matmul(
                            av,
                            lhsT=vh_sb[:, kt, h, :],
                            rhs=st_blk[:, kt, :],
                            start=(kt == 0),
                            stop=(kt == KT - 1),
                        )
                    po = (h % 2) * 64
                    nc.vector.tensor_copy(
                        out=UT_sb[po:po + 64, hp, qc * 512:(qc + 1) * 512],
                        in_=av[0:64, :],
                    )

        # ---- tail: recip rows -> DRAM -> broadcast -> normalize UT, then project ----
        ps_av.release()
        ps_tail = tc.alloc_tile_pool(name="ps_tail", bufs=1, space="PSUM")
        pst = ps_tail.tile([64, 128], DT, tag="ps_t2")
        nc.tensor.transpose(pst, rcall_sb[:, :], ident)
        rrow_sb = small.tile([64, 128], DT, tag="rrow")
        nc.vector.tensor_copy(out=rrow_sb, in_=pst)
        nc.sync.dma_start(out=recip_dram[:, :], in_=rrow_sb[:, :])
        for hp in range(2):
            for hh in range(2):
                h = 2 * hp + hh
                nc.gpsimd.dma_start(
                    out=rb_sb[hh * 64:hh * 64 + 64, hp, :],
                    in_=recip_dram[h * QT:(h + 1) * QT, :].rearrange("a b -> (a b)")[None, :].to_broadcast([64, S]),
                )
            nc.vector.tensor_mul(
                out=UT_sb[:, hp, :],
                in0=UT_sb[:, hp, :],
                in1=rb_sb[:, hp, :],
            )
        for qt in range(QT):
            ps = ps_big.tile([128, 2, 512], F32, tag="ps_big")
            for ec in range(2):
                for cc in range(2):
                    nc.tensor.matmul(
                        ps[:, ec, :],
                        lhsT=UT_sb[:, cc, qt * 128:(qt + 1) * 128],
                        rhs=woT_sb[:, cc, ec * 512:(ec + 1) * 512],
                        start=(cc == 0),
                        stop=(cc == 1),
                    )
            o_sb = o_pool.tile([128, 2, 512], DT, tag="o")
            nc.vector.tensor_copy(out=o_sb, in_=ps)
            nc.sync.dma_start(out=o_out[qt * 128:(qt + 1) * 128, :], in_=o_sb[:, :, :])

        ps_tail.release()
        for p in (small, o_pool, w_pool, st_pool, ps_big, acts, consts):
            p.release()

    nc.finalize()
    return nc


def _get_nc():
    if "nc" not in _CACHE:
        _CACHE["nc"] = _build_nc()
    return _CACHE["nc"]


def _make_in_maps(q, k, v, Wq, bq, Wk, bk, Wv, bv, Wo, bo):
    in_maps = []
    perb = []
    for b in range(B):
        perb.append((
            np.ascontiguousarray(q[b].T).astype(BF),
            np.ascontiguousarray(k[b].T).astype(BF),
            np.ascontiguousarray(v[b].T).astype(BF),
        ))
    for c in range(NCORES):
        b, g = divmod(c, HPG)
        cs = slice(g * CS, (g + 1) * CS)
        qTb, kTb, vTb = perb[b]
        bias = np.zeros((128, 16), np.float32)
        for col0, vec in ((0, bq[cs] * SCALE), (4, bk[cs])):
            for h in range(HPG):
                bias[:64, col0 + h] = vec[h * 64:(h + 1) * 64]
        bias[:, 12] = bv[cs][:128]
        bias[:, 13] = bv[cs][128:]
        in_maps.append({
            "qT": qTb,
            "kT": kTb,
            "vT": vTb,
            "wqT": np.ascontiguousarray((Wq[cs] * SCALE).T).astype(BF),
            "wkT": np.ascontiguousarray(Wk[cs].T).astype(BF),
            "wvT": np.ascontiguousarray(Wv[cs].T).astype(BF),
            "woT": np.ascontiguousarray(Wo[:, cs].T).astype(BF),
            "bias": bias,
        })
    return in_maps


def kernel(q, k, v, Wq, bq, Wk, bk, Wv, bv, Wo, bo, _want_results=False, **_unused):
    from concourse.bass_utils import run_bass_kernel_spmd

    q, k, v = (np.asarray(x, np.float32) for x in (q, k, v))
    Wq, bq, Wk, bk, Wv, bv, Wo, bo = (
        np.asarray(x, np.float32) for x in (Wq, bq, Wk, bk, Wv, bv, Wo, bo)
    )

    nc = _get_nc()
    in_maps = _make_in_maps(q, k, v, Wq, bq, Wk, bk, Wv, bv, Wo, bo)
    res = run_bass_kernel_spmd(nc, in_maps, core_ids=list(range(NCORES)))
    results = res.results

    out = np.zeros((B, S, E), np.float32)
    weights = np.empty((B, H, S, S), np.float32)
    for c in range(NCORES):
        b, g = divmod(c, HPG)
        out[b] += results[c]["o_out"].astype(np.float32)
        weights[b, g * HPG:(g + 1) * HPG] = results[c]["w_out"].astype(np.float32)
    out += bo
    if _want_results:
        return (out, weights), res
    return out, weights
